# revision 5
# baseline (speedup 1.0000x reference)
"""Trainium2 Bass kernel for nn_ChordalPCWeightTransform.

Math: the reference does
    out = softmax( P_orig( P_rootfirst(x) * w ), axis=-1 )
where P_rootfirst / P_orig are per-label rolls of the first 12 pitch
classes (last slot fixed).  The two permutations are exact inverses, so
the whole transform collapses to
    out[b, l, :] = softmax( x[b, l, :] * W[l, :] )
with W[l, j] = w[(j - root_pc(l)) % 12] for j < 12 and W[l, 12] = w[12].
W ([144, 13]) is a cheap host-side gather of the 13 learned weights.

Kernel: pure data parallel over the batch axis (65536 frames -> 8192 per
core x 8 cores).  Per core, 64 tiles of [128 frames, 1872 floats]:
  DMA in (HWDGE/sync) -> gpsimd: t = x * W  -> ACT: e = exp(t)
  -> DVE: s = segmented sum over 13, r ~= 1/s, out = e * r (broadcast)
  -> DMA out (HWDGE/scalar).
Memory-bound: ~123 MB of HBM traffic per core.
"""

import numpy as np

import concourse.bass as bass
import concourse.bacc as bacc
import concourse.tile as tile
from concourse import mybir
from concourse.bass_utils import run_bass_kernel_spmd

B, L, P = 65536, 144, 13
NCORES = 8
BS = B // NCORES  # 8192 frames per core
ROW = L * P       # 1872 floats per frame
TP = 128          # frames per tile (SBUF partitions)

F32 = mybir.dt.float32


def _build_weight_table(w: np.ndarray) -> np.ndarray:
    """Effective per-label weight table W[l, j] = w[idx_original[l, j]]."""
    num_quality = L // 12
    root_pc = np.arange(L) // num_quality
    n = P - 1
    j = np.arange(n)
    idx12 = (j[None, :] - root_pc[:, None]) % n
    idx = np.concatenate([idx12, np.full((L, 1), n, dtype=idx12.dtype)], axis=1)
    return np.ascontiguousarray(w.astype(np.float32)[idx])  # [144, 13]


def build_module(n_frames: int = BS) -> bass.Bass:
    assert n_frames % TP == 0
    nt = n_frames // TP
    nc = bacc.Bacc()
    x_in = nc.declare_dram_parameter("x", [n_frames, ROW], F32, isOutput=False)
    w_in = nc.declare_dram_parameter("w", [ROW], F32, isOutput=False)
    y_out = nc.declare_dram_parameter("y", [n_frames, ROW], F32, isOutput=True)

    with tile.TileContext(nc) as tc:
        with (
            tc.tile_pool(name="singles", bufs=1) as singles,
            tc.tile_pool(name="xin", bufs=4) as xpool,
            tc.tile_pool(name="etile", bufs=4) as epool,
            tc.tile_pool(name="stats", bufs=8) as spool,
        ):
            # Replicate the 1872-float weight row across all 128 partitions.
            wb = singles.tile([TP, ROW], F32)
            nc.gpsimd.dma_start(
                out=wb[:], in_=w_in[None, :].to_broadcast([TP, ROW])
            )

            for i in range(nt):
                rows = slice(i * TP, (i + 1) * TP)

                x_t = xpool.tile([TP, ROW], F32)
                nc.sync.dma_start(out=x_t[:], in_=x_in[rows, :])

                # t = x * W   (gpsimd, keeps DVE under the DMA roofline)
                nc.gpsimd.tensor_tensor(
                    out=x_t[:], in0=x_t[:], in1=wb[:], op=mybir.AluOpType.mult
                )

                # e = exp(t)  (scalar engine LUT)
                e_t = epool.tile([TP, ROW], F32)
                nc.scalar.activation(
                    out=e_t[:], in_=x_t[:],
                    func=mybir.ActivationFunctionType.Exp,
                )

                # s[p, l] = sum_j e[p, l, j]
                e3 = e_t.rearrange("p (g d) -> p g d", d=P)
                s_t = spool.tile([TP, L], F32)
                nc.vector.reduce_sum(
                    out=s_t[:], in_=e3, axis=mybir.AxisListType.X
                )

                # ls = ln(s)   (same ACT table set as exp)
                nc.scalar.activation(
                    out=s_t[:], in_=s_t[:],
                    func=mybir.ActivationFunctionType.Ln,
                )

                # d = t - ls  (broadcast ls over the 13 pitch classes)
                x3 = x_t.rearrange("p (g d) -> p g d", d=P)
                nc.vector.tensor_tensor(
                    out=x3, in0=x3,
                    in1=s_t[:, :, None].to_broadcast([TP, L, P]),
                    op=mybir.AluOpType.subtract,
                )

                # out = exp(d) = softmax(t)
                nc.scalar.activation(
                    out=e_t[:], in_=x_t[:],
                    func=mybir.ActivationFunctionType.Exp,
                )

                nc.scalar.dma_start(out=y_out[rows, :], in_=e_t[:])

    nc.finalize()
    return nc


_MODULE_CACHE: dict[int, bass.Bass] = {}


def _get_module(n_frames: int = BS) -> bass.Bass:
    if n_frames not in _MODULE_CACHE:
        _MODULE_CACHE[n_frames] = build_module(n_frames)
    return _MODULE_CACHE[n_frames]


def make_in_maps(x: np.ndarray, w: np.ndarray) -> list[dict[str, np.ndarray]]:
    weff = _build_weight_table(w).reshape(ROW)
    return [
        {
            "x": np.ascontiguousarray(
                x[i * BS : (i + 1) * BS].reshape(BS, ROW), dtype=np.float32
            ),
            "w": weff,
        }
        for i in range(NCORES)
    ]


def kernel(**inputs: np.ndarray) -> np.ndarray:
    x = np.asarray(inputs["chordal_pc_vector"], dtype=np.float32)
    w = np.asarray(inputs["scale_degree_weight"], dtype=np.float32)
    assert x.shape == (B, L, P), x.shape

    nc = _get_module()
    in_maps = make_in_maps(x, w)
    res = run_bass_kernel_spmd(nc, in_maps, core_ids=list(range(NCORES)))
    out = np.concatenate(
        [res.results[i]["y"].reshape(BS, L, P) for i in range(NCORES)], axis=0
    )
    return out


# revision 7
# speedup vs baseline: 1.6666x; 1.6666x over previous
"""Trainium2 Bass kernel for nn_ChordalPCWeightTransform.

Math: the reference does
    out = softmax( P_orig( P_rootfirst(x) * w ), axis=-1 )
where P_rootfirst / P_orig are per-label rolls of the first 12 pitch
classes (last slot fixed).  The two permutations are exact inverses, so
the whole transform collapses to
    out[b, l, :] = softmax( x[b, l, :] * W[l, :] )
with W[l, j] = w[(j - root_pc(l)) % 12] for j < 12 and W[l, 12] = w[12].
W ([144, 13]) is a cheap host-side gather of the 13 learned weights.

Kernel: pure data parallel over the batch axis (65536 frames -> 8192 per
core x 8 cores).  Per core, 64 tiles of [128 frames, 1872 floats]:
  DMA in (HWDGE/sync) -> gpsimd: t = x * W  -> ACT: e = exp(t)
  -> DVE: s = segmented sum over 13, r ~= 1/s, out = e * r (broadcast)
  -> DMA out (HWDGE/scalar).
Memory-bound: ~123 MB of HBM traffic per core.
"""

import numpy as np

import concourse.bass as bass
import concourse.bacc as bacc
import concourse.tile as tile
from concourse import mybir
from concourse.bass_utils import run_bass_kernel_spmd

B, L, P = 65536, 144, 13
NCORES = 8
BS = B // NCORES  # 8192 frames per core
ROW = L * P       # 1872 floats per frame
TP = 128          # frames per tile (SBUF partitions)

F32 = mybir.dt.float32


def _build_weight_table(w: np.ndarray) -> np.ndarray:
    """Effective per-label weight table W[l, j] = w[idx_original[l, j]]."""
    num_quality = L // 12
    root_pc = np.arange(L) // num_quality
    n = P - 1
    j = np.arange(n)
    idx12 = (j[None, :] - root_pc[:, None]) % n
    idx = np.concatenate([idx12, np.full((L, 1), n, dtype=idx12.dtype)], axis=1)
    return np.ascontiguousarray(w.astype(np.float32)[idx])  # [144, 13]


def _pin_act_table(nc) -> None:
    """Make Exp and Ln resolvable only from the combined set so Bacc emits a
    single ACT_TABLE_LOAD instead of thrashing exp<->ln sets every tile.
    Mutates set contents only -- names/order (= act_func_set_id) unchanged."""
    from concourse.hw_specs import get_activation_tables

    tabs = get_activation_tables(nc.m.arch)
    keep = "natural_log_exp_and_others"
    if keep not in tabs:
        return
    exp = mybir.ActivationFunctionType.Exp
    ln = mybir.ActivationFunctionType.Ln
    for name, fns in tabs.items():
        if name != keep:
            fns.discard(exp)
            fns.discard(ln)


def build_module(n_frames: int = BS) -> bass.Bass:
    assert n_frames % TP == 0
    nt = n_frames // TP
    nc = bacc.Bacc()
    _pin_act_table(nc)
    x_in = nc.declare_dram_parameter("x", [n_frames, ROW], F32, isOutput=False)
    w_in = nc.declare_dram_parameter("w", [ROW], F32, isOutput=False)
    y_out = nc.declare_dram_parameter("y", [n_frames, ROW], F32, isOutput=True)

    with tile.TileContext(nc) as tc:
        with (
            tc.tile_pool(name="singles", bufs=1) as singles,
            tc.tile_pool(name="xin", bufs=6) as xpool,
            tc.tile_pool(name="etile", bufs=6) as epool,
            tc.tile_pool(name="stats", bufs=8) as spool,
        ):
            # Replicate the 1872-float weight row across all 128 partitions.
            wb = singles.tile([TP, ROW], F32)
            nc.gpsimd.dma_start(
                out=wb[:], in_=w_in[None, :].to_broadcast([TP, ROW])
            )

            for i in range(nt):
                rows = slice(i * TP, (i + 1) * TP)

                x_t = xpool.tile([TP, ROW], F32)
                nc.sync.dma_start(out=x_t[:], in_=x_in[rows, :])

                # t = x * W   (gpsimd, keeps DVE under the DMA roofline)
                nc.gpsimd.tensor_tensor(
                    out=x_t[:], in0=x_t[:], in1=wb[:], op=mybir.AluOpType.mult
                )

                # e = exp(t)  (scalar engine LUT)
                e_t = epool.tile([TP, ROW], F32)
                nc.scalar.activation(
                    out=e_t[:], in_=x_t[:],
                    func=mybir.ActivationFunctionType.Exp,
                )

                # s[p, l] = sum_j e[p, l, j]
                e3 = e_t.rearrange("p (g d) -> p g d", d=P)
                s_t = spool.tile([TP, L], F32)
                nc.vector.reduce_sum(
                    out=s_t[:], in_=e3, axis=mybir.AxisListType.X
                )

                # ls = ln(s)   (same ACT table set as exp)
                nc.scalar.activation(
                    out=s_t[:], in_=s_t[:],
                    func=mybir.ActivationFunctionType.Ln,
                )

                # d = t - ls  (broadcast ls over the 13 pitch classes)
                x3 = x_t.rearrange("p (g d) -> p g d", d=P)
                nc.vector.tensor_tensor(
                    out=x3, in0=x3,
                    in1=s_t[:, :, None].to_broadcast([TP, L, P]),
                    op=mybir.AluOpType.subtract,
                )

                # out = exp(d) = softmax(t)
                nc.scalar.activation(
                    out=e_t[:], in_=x_t[:],
                    func=mybir.ActivationFunctionType.Exp,
                )

                nc.scalar.dma_start(out=y_out[rows, :], in_=e_t[:])

    nc.finalize()
    return nc


_MODULE_CACHE: dict[int, bass.Bass] = {}


def _get_module(n_frames: int = BS) -> bass.Bass:
    if n_frames not in _MODULE_CACHE:
        _MODULE_CACHE[n_frames] = build_module(n_frames)
    return _MODULE_CACHE[n_frames]


def make_in_maps(x: np.ndarray, w: np.ndarray) -> list[dict[str, np.ndarray]]:
    weff = _build_weight_table(w).reshape(ROW)
    return [
        {
            "x": np.ascontiguousarray(
                x[i * BS : (i + 1) * BS].reshape(BS, ROW), dtype=np.float32
            ),
            "w": weff,
        }
        for i in range(NCORES)
    ]


def kernel(**inputs: np.ndarray) -> np.ndarray:
    x = np.asarray(inputs["chordal_pc_vector"], dtype=np.float32)
    w = np.asarray(inputs["scale_degree_weight"], dtype=np.float32)
    assert x.shape == (B, L, P), x.shape

    nc = _get_module()
    in_maps = make_in_maps(x, w)
    res = run_bass_kernel_spmd(nc, in_maps, core_ids=list(range(NCORES)))
    out = np.concatenate(
        [res.results[i]["y"].reshape(BS, L, P) for i in range(NCORES)], axis=0
    )
    return out


# revision 9
# speedup vs baseline: 1.8063x; 1.0838x over previous
"""Trainium2 Bass kernel for nn_ChordalPCWeightTransform.

Math: the reference does
    out = softmax( P_orig( P_rootfirst(x) * w ), axis=-1 )
where P_rootfirst / P_orig are per-label rolls of the first 12 pitch
classes (last slot fixed).  The two permutations are exact inverses, so
the whole transform collapses to
    out[b, l, :] = softmax( x[b, l, :] * W[l, :] )
with W[l, j] = w[(j - root_pc(l)) % 12] for j < 12 and W[l, 12] = w[12].
W ([144, 13]) is a cheap host-side gather of the 13 learned weights.

Kernel: pure data parallel over the batch axis (65536 frames -> 8192 per
core x 8 cores).  Per core, 64 tiles of [128 frames, 1872 floats]:
  DMA in (HWDGE/sync) -> gpsimd: t = x * W  -> ACT: e = exp(t)
  -> DVE: s = segmented sum over 13, r ~= 1/s, out = e * r (broadcast)
  -> DMA out (HWDGE/scalar).
Memory-bound: ~123 MB of HBM traffic per core.
"""

import numpy as np

import concourse.bass as bass
import concourse.bacc as bacc
import concourse.tile as tile
from concourse import mybir
from concourse.bass_utils import run_bass_kernel_spmd

B, L, P = 65536, 144, 13
NCORES = 8
BS = B // NCORES  # 8192 frames per core
ROW = L * P       # 1872 floats per frame
TP = 128          # frames per tile (SBUF partitions)

F32 = mybir.dt.float32


def _build_weight_table(w: np.ndarray) -> np.ndarray:
    """Effective per-label weight table W[l, j] = w[idx_original[l, j]]."""
    num_quality = L // 12
    root_pc = np.arange(L) // num_quality
    n = P - 1
    j = np.arange(n)
    idx12 = (j[None, :] - root_pc[:, None]) % n
    idx = np.concatenate([idx12, np.full((L, 1), n, dtype=idx12.dtype)], axis=1)
    return np.ascontiguousarray(w.astype(np.float32)[idx])  # [144, 13]


def _pin_act_table(nc) -> None:
    """Make Exp and Ln resolvable only from the combined set so Bacc emits a
    single ACT_TABLE_LOAD instead of thrashing exp<->ln sets every tile.
    Mutates set contents only -- names/order (= act_func_set_id) unchanged."""
    from concourse.hw_specs import get_activation_tables

    tabs = get_activation_tables(nc.m.arch)
    keep = "natural_log_exp_and_others"
    if keep not in tabs:
        return
    exp = mybir.ActivationFunctionType.Exp
    ln = mybir.ActivationFunctionType.Ln
    for name, fns in tabs.items():
        if name != keep:
            fns.discard(exp)
            fns.discard(ln)


def build_module(n_frames: int = BS) -> bass.Bass:
    assert n_frames % TP == 0
    nt = n_frames // TP
    nc = bacc.Bacc()
    _pin_act_table(nc)
    x_in = nc.declare_dram_parameter("x", [n_frames, ROW], F32, isOutput=False)
    w_in = nc.declare_dram_parameter("w", [ROW], F32, isOutput=False)
    y_out = nc.declare_dram_parameter("y", [n_frames, ROW], F32, isOutput=True)

    with tile.TileContext(nc) as tc:
        with (
            tc.tile_pool(name="singles", bufs=1) as singles,
            tc.tile_pool(name="xin", bufs=6) as xpool,
            tc.tile_pool(name="etile", bufs=6) as epool,
            tc.tile_pool(name="stats", bufs=7, space="PSUM") as spool,
        ):
            # Replicate the 1872-float weight row across all 128 partitions.
            wb = singles.tile([TP, ROW], F32)
            nc.gpsimd.dma_start(
                out=wb[:], in_=w_in[None, :].to_broadcast([TP, ROW])
            )

            for i in range(nt):
                rows = slice(i * TP, (i + 1) * TP)

                x_t = xpool.tile([TP, ROW], F32)
                nc.sync.dma_start(out=x_t[:], in_=x_in[rows, :])

                # t = x * W on gpsimd.  DVE ops below are kept 1-port on
                # SBUF (stats live in PSUM) so they don't contend with
                # gpsimd for the shared SBUF port.
                nc.gpsimd.tensor_tensor(
                    out=x_t[:], in0=x_t[:], in1=wb[:], op=mybir.AluOpType.mult
                )

                # e = exp(t)  (scalar engine LUT)
                e_t = epool.tile([TP, ROW], F32)
                nc.scalar.activation(
                    out=e_t[:], in_=x_t[:],
                    func=mybir.ActivationFunctionType.Exp,
                )

                # s[p, l] = sum_j e[p, l, j]  (DVE, SBUF rd0 -> PSUM)
                e3 = e_t.rearrange("p (g d) -> p g d", d=P)
                s_t = spool.tile([TP, L], F32)
                nc.vector.reduce_sum(
                    out=s_t[:], in_=e3, axis=mybir.AxisListType.X
                )

                # ls = ln(s)   (ACT, PSUM -> PSUM; same table set as exp)
                nc.scalar.activation(
                    out=s_t[:], in_=s_t[:],
                    func=mybir.ActivationFunctionType.Ln,
                )

                # d = t - ls  (DVE: in0 SBUF rd0, in1 PSUM port, out SBUF)
                x3 = x_t.rearrange("p (g d) -> p g d", d=P)
                nc.vector.tensor_tensor(
                    out=x3, in0=x3,
                    in1=s_t[:, :, None].to_broadcast([TP, L, P]),
                    op=mybir.AluOpType.subtract,
                )

                # out = exp(d) = softmax(t)
                nc.scalar.activation(
                    out=e_t[:], in_=x_t[:],
                    func=mybir.ActivationFunctionType.Exp,
                )

                nc.scalar.dma_start(out=y_out[rows, :], in_=e_t[:])

    nc.finalize()
    return nc


_MODULE_CACHE: dict[int, bass.Bass] = {}


def _get_module(n_frames: int = BS) -> bass.Bass:
    if n_frames not in _MODULE_CACHE:
        _MODULE_CACHE[n_frames] = build_module(n_frames)
    return _MODULE_CACHE[n_frames]


def make_in_maps(x: np.ndarray, w: np.ndarray) -> list[dict[str, np.ndarray]]:
    weff = _build_weight_table(w).reshape(ROW)
    return [
        {
            "x": np.ascontiguousarray(
                x[i * BS : (i + 1) * BS].reshape(BS, ROW), dtype=np.float32
            ),
            "w": weff,
        }
        for i in range(NCORES)
    ]


def kernel(**inputs: np.ndarray) -> np.ndarray:
    x = np.asarray(inputs["chordal_pc_vector"], dtype=np.float32)
    w = np.asarray(inputs["scale_degree_weight"], dtype=np.float32)
    assert x.shape == (B, L, P), x.shape

    nc = _get_module()
    in_maps = make_in_maps(x, w)
    res = run_bass_kernel_spmd(nc, in_maps, core_ids=list(range(NCORES)))
    out = np.concatenate(
        [res.results[i]["y"].reshape(BS, L, P) for i in range(NCORES)], axis=0
    )
    return out
